# revision 22
# baseline (speedup 1.0000x reference)
"""MoE experts (32 experts, top-2, SwiGLU MLP) on 8 trn2 NeuronCores.

Expert-parallel sharding: core c owns 4 experts. Routing metadata is
computed on host from top_k_indices; each core receives its experts'
weights (pre-transposed to matmul layout) plus the dispatched token
activations, runs the grouped SwiGLU MLP on device (fp16 operands, fp32
accumulation) and returns per-slot outputs in fp16. Host scatters
per-slot outputs back to (token, k), applies the routing weights, and
sums over the top-k axis (the expert-parallel combine/unshard).

Schedule notes (v2):
- All inputs are SBUF-resident (no tile recycling): every load DMA is
  issued up front so the input stream runs back-to-back at full HBM BW.
- The startup-critical first chunks (expert 0's tokens + first w1
  block) ride the Pool-engine DMA queue, whose DGE config is ~25 ns vs
  ~600 ns on SP; everything else streams FIFO on the SP queue in exact
  consumption order. Output stores ride the Pool queue too.
- w1 is laid out it-major (blk = it*8 + role*4 + ht) and w2 ht2-major
  (blk = ht2*8 + it) so consumption is a linear prefix scan and early
  matmuls only wait on early bytes.
- ~18 junk matmuls pinned at program start warm the PE p-state ramp
  (1.2 GHz for the first ~3 us of busy time) before real data lands.
- Routing-weight scaling + top-k combine happen on host; the device
  only runs the grouped MLP and stores y in fp16.
"""

import sys
import types

import numpy as np

# Model dims (hardcoded per problem spec nn_MoEExperts_27109833572673)
T, TOPK, E, H, I = 4096, 2, 32, 512, 1024
CAP = 2 * (T * TOPK) // E  # 512
NCORES = 8
EPC = E // NCORES  # experts per core = 4
HT = H // 128  # 4 h-tiles
IT = I // 128  # 8 i-tiles

N_WARMUP_MM = 28  # 128-col dummy matmuls to cover the PE p-state ramp

LAST_RESULTS = None  # BassKernelResults of the most recent device run


def _ensure_profile_hook():
    """Register the NTFF profile hook if the env lacks antenv.axon_hooks.

    Only needed when tracing (BASS_TRACE=1 / trace=True); safe no-op
    otherwise. Mirrors trn_agent_boot.trn_boot step 6.
    """
    try:
        if "antenv.axon_hooks" in sys.modules:
            return
        import antenv

        mod = types.ModuleType("antenv.axon_hooks")
        state = {"hook": None}
        mod.set_axon_ntff_profile_hook = lambda h: state.__setitem__("hook", h)
        mod.get_axon_ntff_profile_hook = lambda: state["hook"]
        sys.modules["antenv.axon_hooks"] = mod
        antenv.axon_hooks = mod
        try:
            from trn_agent_boot.trn_boot import _ntff_profile_via_ctypes

            mod.set_axon_ntff_profile_hook(
                _ntff_profile_via_ctypes("/opt/axon/libaxon_pjrt.so")
            )
        except Exception:
            pass
    except Exception:
        pass


def _routing(top_k_indices, top_k_weights):
    """Per-expert slot lists (ascending flat order == Switch dispatch pos),
    clipped at CAP exactly like the reference's capacity drop."""
    e_flat = np.asarray(top_k_indices).reshape(-1).astype(np.int32)
    w_flat = np.asarray(top_k_weights).reshape(-1).astype(np.float32)
    tok = np.arange(T * TOPK, dtype=np.int32) // TOPK
    order = np.argsort(e_flat, kind="stable")
    sorted_e = e_flat[order]
    starts = np.searchsorted(sorted_e, np.arange(E + 1))
    slots_per_e = [order[starts[e] : starts[e + 1]][:CAP] for e in range(E)]
    return e_flat, w_flat, tok, slots_per_e


_prog_cache = {}


def _build_program(m_pads):
    """One SPMD program: per-core grouped SwiGLU MLP over EPC experts,
    position j padded to m_pads[j] slots."""
    import concourse.bacc as bacc
    import concourse.mybir as mybir
    from concourse.tile import TileContext

    f32 = mybir.dt.float32
    f16 = mybir.dt.float16
    slots = int(sum(m_pads))
    offs = [0]
    for m in m_pads:
        offs.append(offs[-1] + int(m))

    nc = bacc.Bacc("TRN2", target_bir_lowering=False, debug=False,
                   num_devices=NCORES)
    # Host lays every input out so each device DMA is one plain [128, X]
    # copy in exact consumption order:
    #   xdT[p, HT*off_j + ht*m_j + s]
    #   w1n[j, p, (it*8 + role*4 + ht)*128 + o]   (role 0=gate, 1=up)
    #   w2n[j, p, (ht2*8 + it)*128 + h]
    #   y[p, HT*off_j + ht2*m_j + s]  (fp16)
    xdT_d = nc.declare_dram_parameter("xdT", [128, HT * slots], f16,
                                      isOutput=False)
    w1n_d = nc.declare_dram_parameter("w1n", [EPC, 128, 64 * 128], f16,
                                      isOutput=False)
    w2n_d = nc.declare_dram_parameter("w2n", [EPC, 128, 32 * 128], f16,
                                      isOutput=False)
    y_d = nc.declare_dram_parameter("y", [128, HT * slots], f16,
                                    isOutput=True)

    with TileContext(nc) as tc:
        with (
            tc.tile_pool(name="res", bufs=1) as resp,
            tc.tile_pool(name="act", bufs=2) as actp,
            tc.tile_pool(name="ps", bufs=2, space="PSUM") as psp,
        ):
            # --- PE p-state warm-up: junk matmuls on a zeroed tile. ---
            with tc.high_priority():
                dum = resp.tile([128, 128], f16, tag="dum", name="dum")
                nc.gpsimd.memset(dum[:], 0.0)
                pdum = psp.tile([128, 128], f32, tag="ps2", name="pdum")
                for k in range(N_WARMUP_MM):
                    nc.tensor.matmul(pdum[:], dum[:], dum[:],
                                     start=(k == 0),
                                     stop=(k == N_WARMUP_MM - 1))

            # --- Resident input tiles + all load DMAs on ONE SP HWDGE
            # ring, in exact consumption order: the FIFO then streams
            # back-to-back at full HBM bandwidth and every chunk lands
            # just before the PE needs it. Chunk granularity is finest
            # for expert 0 (startup critical path) and w2 is split in
            # ht2-halves so mm2 never waits on a whole-megabyte sem.
            m0 = int(m_pads[0])
            xt = [None] * EPC          # per-expert x access: xt[j][ht] -> AP
            w1s = [None] * EPC         # w1s[j](it, role, ht) -> stationary AP
            w2t = [None] * EPC         # w2 tile per expert
            x0 = resp.tile([128, HT * m0], f16, tag="x0", name="x0")
            nc.sync.dma_start(out=x0[:], in_=xdT_d[:, 0 : HT * m0])
            w1_0g = resp.tile([128, 4 * 128], f16, tag="w1_0g", name="w1_0g")
            nc.sync.dma_start(out=w1_0g[:], in_=w1n_d[0, :, 0 : 4 * 128])
            w1_0u = resp.tile([128, 4 * 128], f16, tag="w1_0u", name="w1_0u")
            nc.sync.dma_start(out=w1_0u[:],
                              in_=w1n_d[0, :, 4 * 128 : 8 * 128])
            xt[0] = [x0[:, ht * m0 : (ht + 1) * m0] for ht in range(HT)]

            # Expert 0's w1 it1..7 as per-it chunks. (All loads stay on
            # the single SP ring: a second HWDGE ring measurably inflates
            # the program preamble and steals stream bandwidth from the
            # startup-critical chunks.)
            w1_0it = [None] * IT
            for it in range(1, IT):
                t = resp.tile([128, 8 * 128], f16, tag=f"w1_0it{it}",
                              name=f"w1_0it{it}")
                nc.sync.dma_start(
                    out=t[:], in_=w1n_d[0, :, it * 8 * 128 : (it + 1) * 8 * 128])
                w1_0it[it] = t

            def w1s_0(it, role, ht):
                if it == 0:
                    t = w1_0g if role == 0 else w1_0u
                    return t[:, ht * 128 : (ht + 1) * 128]
                b = (role * 4 + ht) * 128
                return w1_0it[it][:, b : b + 128]

            w1s[0] = w1s_0

            def load_w2(j, nchunks=2):
                t = resp.tile([128, 32 * 128], f16, tag=f"w2_{j}",
                              name=f"w2_{j}")
                step = 32 * 128 // nchunks
                for k in range(nchunks):
                    nc.sync.dma_start(out=t[:, k * step : (k + 1) * step],
                                      in_=w2n_d[j, :, k * step : (k + 1) * step])
                return t

            def load_rest(j, head_blks):
                """x_j, then w1_j chunked per head_blks (block counts),
                finer at the front so mm1_j's first groups never wait on
                a half-megabyte completion semaphore."""
                m = int(m_pads[j])
                xj = resp.tile([128, HT * m], f16, tag=f"x{j}", name=f"x{j}")
                nc.sync.dma_start(
                    out=xj[:],
                    in_=xdT_d[:, HT * offs[j] : HT * offs[j] + HT * m])
                xt[j] = [xj[:, ht * m : (ht + 1) * m] for ht in range(HT)]
                chunks = []   # (start_blk, tile)
                b0 = 0
                for k, nb in enumerate(head_blks):
                    c = resp.tile([128, nb * 128], f16, tag=f"w1_{j}c{k}",
                                  name=f"w1_{j}c{k}")
                    nc.sync.dma_start(
                        out=c[:],
                        in_=w1n_d[j, :, b0 * 128 : (b0 + nb) * 128])
                    chunks.append((b0, c))
                    b0 += nb
                assert b0 == 64

                def w1s_j(it, role, ht, _chunks=chunks):
                    blk = it * 8 + role * 4 + ht
                    for s, c in reversed(_chunks):
                        if blk >= s:
                            return c[:, (blk - s) * 128 : (blk - s + 1) * 128]

                w1s[j] = w1s_j

            # Consumption-ordered remainder: w2_0 right after expert 0's
            # w1 (mm2_0 follows mm1_0), then x/w1/w2 per expert.
            # (Finer head chunks were tried and regress: every extra
            # dma_start adds ~0.6us of serial DGE config at the stream
            # head, which outweighs the tighter arrival tracking.)
            w2t[0] = load_w2(0)
            for j in range(1, EPC):
                load_rest(j, (16, 16, 16, 16))
                w2t[j] = load_w2(j)

            # --- Compute: grouped SwiGLU MLP per expert. ---
            for j in range(EPC):
                m = int(m_pads[j])
                # mm1: out1^T[o, s] = sum_h W1[o, h] * xd[s, h] per o-tile.
                acts = []
                for it in range(IT):
                    pg = psp.tile([128, m], f32, tag="pg", name="pg", bufs=3)
                    pu = psp.tile([128, m], f32, tag="pu", name="pu", bufs=3)
                    for ht in range(HT):
                        nc.tensor.matmul(pg[:], w1s[j](it, 0, ht), xt[j][ht],
                                         start=(ht == 0), stop=(ht == HT - 1))
                    for ht in range(HT):
                        nc.tensor.matmul(pu[:], w1s[j](it, 1, ht), xt[j][ht],
                                         start=(ht == 0), stop=(ht == HT - 1))
                    sg = actp.tile([128, m], f16, tag="sg", name="sg")
                    nc.scalar.activation(sg[:], pg[:],
                                         mybir.ActivationFunctionType.Silu)
                    a = actp.tile([128, m], f16, tag=f"a{it}", name=f"a{it}")
                    nc.vector.tensor_mul(a[:], sg[:], pu[:])
                    acts.append(a)

                # mm2: y^T[h, s] = sum_i W2[h, i] * act[s, i]; psum copied
                # to the fp16 output tile, alternating Vector/Scalar
                # (Scalar takes the last chunk — it's slightly faster and
                # the final store gates the teardown). Stores ride the
                # same SP ring; their FIFO entries queue up behind the
                # remaining input stream, which is harmless.
                yj = resp.tile([128, HT * m], f16, tag=f"y{j}", name=f"y{j}")
                ybase = HT * offs[j]
                for ht2 in range(HT):
                    ps2 = psp.tile([128, m], f32, tag="ps2", name="ps2")
                    for it in range(IT):
                        b2 = (ht2 * 8 + it) * 128
                        nc.tensor.matmul(ps2[:], w2t[j][:, b2 : b2 + 128],
                                         acts[it][:],
                                         start=(it == 0), stop=(it == IT - 1))
                    dst = yj[:, ht2 * m : (ht2 + 1) * m]
                    if j == EPC - 1 and ht2 == HT - 1:
                        # Final chunk: split the psum copy across both
                        # engines — it gates the last store and teardown.
                        h = m // 2
                        nc.vector.tensor_scalar_mul(dst[:, :h], ps2[:, :h],
                                                    1.0)
                        nc.scalar.copy(dst[:, h:], ps2[:, h:])
                    elif ht2 % 2 == 0:
                        nc.vector.tensor_scalar_mul(dst, ps2[:], 1.0)
                    else:
                        nc.scalar.copy(dst, ps2[:])
                    if j == EPC - 1 and ht2 == HT - 1:
                        # Final chunk in two halves so the very last
                        # store transfer is minimal.
                        h = m // 2
                        nc.sync.dma_start(
                            out=y_d[:, ybase + ht2 * m : ybase + ht2 * m + h],
                            in_=dst[:, :h])
                        nc.sync.dma_start(
                            out=y_d[:, ybase + ht2 * m + h
                                    : ybase + (ht2 + 1) * m],
                            in_=dst[:, h:])
                    elif j == EPC - 1:
                        # Last expert: store each quarter as soon as its
                        # copy lands, so the final store is tiny.
                        nc.sync.dma_start(
                            out=y_d[:, ybase + ht2 * m : ybase + (ht2 + 1) * m],
                            in_=dst)
                if j < EPC - 1:
                    nc.sync.dma_start(
                        out=y_d[:, ybase : ybase + HT * m], in_=yj[:])

    nc.finalize()
    return nc


def kernel(hidden_states, top_k_indices, top_k_weights, gate_up_proj,
           down_proj):
    global LAST_RESULTS
    _ensure_profile_hook()
    from concourse.bass_utils import run_bass_kernel_spmd

    hs = np.ascontiguousarray(np.asarray(hidden_states, dtype=np.float32))
    gup = np.asarray(gate_up_proj, dtype=np.float32)
    dwn = np.asarray(down_proj, dtype=np.float32)

    e_flat, w_flat, tok, slots_per_e = _routing(top_k_indices, top_k_weights)
    counts = np.array([len(s) for s in slots_per_e])
    # Load-balance: sort experts by routed count and deal them out in
    # rounds of NCORES — position j on every core handles one expert from
    # round j, so the per-position compile-time pad (the round max) stays
    # as tight as possible. Descending order also puts the smallest
    # expert last, shortening the post-stream mm2 tail.
    sorted_eids = np.argsort(-counts, kind="stable")
    assign = sorted_eids.reshape(EPC, NCORES)  # [position, core]
    m_pads = tuple(
        int(min(CAP, max(128, int(counts[assign[j]].max()))))
        for j in range(EPC))
    offs = [0]
    for m in m_pads:
        offs.append(offs[-1] + m)
    slots = offs[-1]

    if m_pads not in _prog_cache:
        _prog_cache[m_pads] = _build_program(m_pads)
    nc = _prog_cache[m_pads]

    in_maps = []
    core_exps = []
    for c in range(NCORES):
        exps = [int(assign[j, c]) for j in range(EPC)]
        core_exps.append(exps)
        xd = np.zeros((slots, H), np.float32)
        for j, e in enumerate(exps):
            sl = slots_per_e[e]
            xd[offs[j] : offs[j] + len(sl)] = hs[tok[sl]]
        # xdT[p, HT*off_j + ht*m_j + s] = xd[off_j + s, ht*128 + p]
        parts = []
        for j in range(EPC):
            blk = xd[offs[j] : offs[j + 1]]  # [m_j, H]
            parts.append(
                blk.reshape(m_pads[j], HT, 128).transpose(2, 1, 0)
                .reshape(128, HT * m_pads[j]))
        xdT = np.ascontiguousarray(np.concatenate(parts, axis=1)
                                   .astype(np.float16))
        # w1n[j, p, (it*8 + role*4 + ht)*128 + o]
        #   = gate_up[e_j, role*I + it*128 + o, ht*128 + p]
        w1n = np.ascontiguousarray(
            gup[exps].reshape(EPC, 2, IT, 128, HT, 128)
            .transpose(0, 5, 2, 1, 4, 3)
            .astype(np.float16)).reshape(EPC, 128, 64 * 128)
        # w2n[j, p, (ht2*8 + it)*128 + h] = down[e_j, ht2*128 + h, it*128 + p]
        w2n = np.ascontiguousarray(
            dwn[exps].reshape(EPC, HT, 128, IT, 128)
            .transpose(0, 4, 1, 3, 2)
            .astype(np.float16)).reshape(EPC, 128, 32 * 128)
        in_maps.append({"xdT": xdT, "w1n": w1n, "w2n": w2n})

    res = run_bass_kernel_spmd(nc, in_maps, list(range(NCORES)))
    LAST_RESULTS = res

    # Combine: scatter per-slot outputs back to flat (token, k) slots,
    # apply the routing weights, and reduce over the top-k axis.
    y_tk = np.zeros((T * TOPK, H), np.float32)
    for c in range(NCORES):
        yc = res.results[c]["y"]  # [128, HT*slots] fp16; y^T[h, s] blocks
        for j, e in enumerate(core_exps[c]):
            sl = slots_per_e[e]
            blk = (yc[:, HT * offs[j] : HT * offs[j + 1]]
                   .reshape(128, HT, m_pads[j]))
            # y[s, h] with h = ht*128 + p
            y_full = (blk.transpose(2, 1, 0).reshape(m_pads[j], H)[: len(sl)]
                      .astype(np.float32))
            y_tk[sl] = y_full * w_flat[sl][:, None]
    return y_tk.reshape(T, TOPK, H).sum(axis=1)


# revision 24
# speedup vs baseline: 1.0100x; 1.0100x over previous
"""MoE experts (32 experts, top-2, SwiGLU MLP) on 8 trn2 NeuronCores.

Expert-parallel sharding: core c owns 4 experts. Routing metadata is
computed on host from top_k_indices; each core receives its experts'
weights (pre-transposed to matmul layout) plus the dispatched token
activations, runs the grouped SwiGLU MLP on device (fp16 operands, fp32
accumulation) and returns per-slot outputs in fp16. Host scatters
per-slot outputs back to (token, k), applies the routing weights, and
sums over the top-k axis (the expert-parallel combine/unshard).

Schedule notes (v2):
- All inputs are SBUF-resident (no tile recycling): every load DMA is
  issued up front so the input stream runs back-to-back at full HBM BW.
- The startup-critical first chunks (expert 0's tokens + first w1
  block) ride the Pool-engine DMA queue, whose DGE config is ~25 ns vs
  ~600 ns on SP; everything else streams FIFO on the SP queue in exact
  consumption order. Output stores ride the Pool queue too.
- w1 is laid out it-major (blk = it*8 + role*4 + ht) and w2 ht2-major
  (blk = ht2*8 + it) so consumption is a linear prefix scan and early
  matmuls only wait on early bytes.
- ~18 junk matmuls pinned at program start warm the PE p-state ramp
  (1.2 GHz for the first ~3 us of busy time) before real data lands.
- Routing-weight scaling + top-k combine happen on host; the device
  only runs the grouped MLP and stores y in fp16.
"""

import sys
import types

import numpy as np

# Model dims (hardcoded per problem spec nn_MoEExperts_27109833572673)
T, TOPK, E, H, I = 4096, 2, 32, 512, 1024
CAP = 2 * (T * TOPK) // E  # 512
NCORES = 8
EPC = E // NCORES  # experts per core = 4
HT = H // 128  # 4 h-tiles
IT = I // 128  # 8 i-tiles

N_WARMUP_MM = 16  # 256-col dummy matmuls to cover the PE p-state ramp

LAST_RESULTS = None  # BassKernelResults of the most recent device run


def _ensure_profile_hook():
    """Register the NTFF profile hook if the env lacks antenv.axon_hooks.

    Only needed when tracing (BASS_TRACE=1 / trace=True); safe no-op
    otherwise. Mirrors trn_agent_boot.trn_boot step 6.
    """
    try:
        if "antenv.axon_hooks" in sys.modules:
            return
        import antenv

        mod = types.ModuleType("antenv.axon_hooks")
        state = {"hook": None}
        mod.set_axon_ntff_profile_hook = lambda h: state.__setitem__("hook", h)
        mod.get_axon_ntff_profile_hook = lambda: state["hook"]
        sys.modules["antenv.axon_hooks"] = mod
        antenv.axon_hooks = mod
        try:
            from trn_agent_boot.trn_boot import _ntff_profile_via_ctypes

            mod.set_axon_ntff_profile_hook(
                _ntff_profile_via_ctypes("/opt/axon/libaxon_pjrt.so")
            )
        except Exception:
            pass
    except Exception:
        pass


def _routing(top_k_indices, top_k_weights):
    """Per-expert slot lists (ascending flat order == Switch dispatch pos),
    clipped at CAP exactly like the reference's capacity drop."""
    e_flat = np.asarray(top_k_indices).reshape(-1).astype(np.int32)
    w_flat = np.asarray(top_k_weights).reshape(-1).astype(np.float32)
    tok = np.arange(T * TOPK, dtype=np.int32) // TOPK
    order = np.argsort(e_flat, kind="stable")
    sorted_e = e_flat[order]
    starts = np.searchsorted(sorted_e, np.arange(E + 1))
    slots_per_e = [order[starts[e] : starts[e + 1]][:CAP] for e in range(E)]
    return e_flat, w_flat, tok, slots_per_e


_prog_cache = {}


def _build_program(m_pads):
    """One SPMD program: per-core grouped SwiGLU MLP over EPC experts,
    position j padded to m_pads[j] slots."""
    import concourse.bacc as bacc
    import concourse.mybir as mybir
    from concourse.tile import TileContext

    f32 = mybir.dt.float32
    f16 = mybir.dt.float16
    slots = int(sum(m_pads))
    offs = [0]
    for m in m_pads:
        offs.append(offs[-1] + int(m))

    nc = bacc.Bacc("TRN2", target_bir_lowering=False, debug=False,
                   num_devices=NCORES)
    # Host lays every input out so each device DMA is one plain [128, X]
    # copy in exact consumption order:
    #   xdT[p, HT*off_j + ht*m_j + s]
    #   w1n[j, p, (it*8 + role*4 + ht)*128 + o]   (role 0=gate, 1=up)
    #   w2n[j, p, (ht2*8 + it)*128 + h]
    #   y[p, HT*off_j + ht2*m_j + s]  (fp16)
    xdT_d = nc.declare_dram_parameter("xdT", [128, HT * slots], f16,
                                      isOutput=False)
    w1n_d = nc.declare_dram_parameter("w1n", [EPC, 128, 64 * 128], f16,
                                      isOutput=False)
    w2n_d = nc.declare_dram_parameter("w2n", [EPC, 128, 32 * 128], f16,
                                      isOutput=False)
    y_d = nc.declare_dram_parameter("y", [128, HT * slots], f16,
                                    isOutput=True)

    with TileContext(nc) as tc:
        with (
            tc.tile_pool(name="res", bufs=1) as resp,
            tc.tile_pool(name="act", bufs=2) as actp,
            tc.tile_pool(name="ps", bufs=2, space="PSUM") as psp,
        ):
            # --- PE p-state warm-up: junk matmuls on a zeroed tile. ---
            with tc.high_priority():
                dum = resp.tile([128, 256], f16, tag="dum", name="dum")
                nc.gpsimd.memset(dum[:], 0.0)
                pdum = psp.tile([128, 256], f32, tag="ps2", name="pdum")
                for k in range(N_WARMUP_MM):
                    nc.tensor.matmul(pdum[:], dum[:, :128], dum[:],
                                     start=(k == 0),
                                     stop=(k == N_WARMUP_MM - 1))

            # --- Resident input tiles + all load DMAs on ONE SP HWDGE
            # ring, in exact consumption order: the FIFO then streams
            # back-to-back at full HBM bandwidth and every chunk lands
            # just before the PE needs it. Chunk granularity is finest
            # for expert 0 (startup critical path) and w2 is split in
            # ht2-halves so mm2 never waits on a whole-megabyte sem.
            m0 = int(m_pads[0])
            xt = [None] * EPC          # per-expert x access: xt[j][ht] -> AP
            w1s = [None] * EPC         # w1s[j](it, role, ht) -> stationary AP
            w2t = [None] * EPC         # w2 tile per expert
            x0 = resp.tile([128, HT * m0], f16, tag="x0", name="x0")
            nc.sync.dma_start(out=x0[:], in_=xdT_d[:, 0 : HT * m0])
            w1_0g = resp.tile([128, 4 * 128], f16, tag="w1_0g", name="w1_0g")
            nc.sync.dma_start(out=w1_0g[:], in_=w1n_d[0, :, 0 : 4 * 128])
            w1_0u = resp.tile([128, 4 * 128], f16, tag="w1_0u", name="w1_0u")
            nc.sync.dma_start(out=w1_0u[:],
                              in_=w1n_d[0, :, 4 * 128 : 8 * 128])
            xt[0] = [x0[:, ht * m0 : (ht + 1) * m0] for ht in range(HT)]

            # Expert 0's w1 it1..7 as per-it chunks. (All loads stay on
            # the single SP ring: a second HWDGE ring measurably inflates
            # the program preamble and steals stream bandwidth from the
            # startup-critical chunks.)
            w1_0it = [None] * IT
            for it in range(1, IT):
                t = resp.tile([128, 8 * 128], f16, tag=f"w1_0it{it}",
                              name=f"w1_0it{it}")
                nc.sync.dma_start(
                    out=t[:], in_=w1n_d[0, :, it * 8 * 128 : (it + 1) * 8 * 128])
                w1_0it[it] = t

            def w1s_0(it, role, ht):
                if it == 0:
                    t = w1_0g if role == 0 else w1_0u
                    return t[:, ht * 128 : (ht + 1) * 128]
                b = (role * 4 + ht) * 128
                return w1_0it[it][:, b : b + 128]

            w1s[0] = w1s_0

            def load_w2(j, nchunks=2):
                t = resp.tile([128, 32 * 128], f16, tag=f"w2_{j}",
                              name=f"w2_{j}")
                step = 32 * 128 // nchunks
                for k in range(nchunks):
                    nc.sync.dma_start(out=t[:, k * step : (k + 1) * step],
                                      in_=w2n_d[j, :, k * step : (k + 1) * step])
                return t

            def load_rest(j, head_blks):
                """x_j, then w1_j chunked per head_blks (block counts),
                finer at the front so mm1_j's first groups never wait on
                a half-megabyte completion semaphore."""
                m = int(m_pads[j])
                xj = resp.tile([128, HT * m], f16, tag=f"x{j}", name=f"x{j}")
                nc.sync.dma_start(
                    out=xj[:],
                    in_=xdT_d[:, HT * offs[j] : HT * offs[j] + HT * m])
                xt[j] = [xj[:, ht * m : (ht + 1) * m] for ht in range(HT)]
                chunks = []   # (start_blk, tile)
                b0 = 0
                for k, nb in enumerate(head_blks):
                    c = resp.tile([128, nb * 128], f16, tag=f"w1_{j}c{k}",
                                  name=f"w1_{j}c{k}")
                    nc.sync.dma_start(
                        out=c[:],
                        in_=w1n_d[j, :, b0 * 128 : (b0 + nb) * 128])
                    chunks.append((b0, c))
                    b0 += nb
                assert b0 == 64

                def w1s_j(it, role, ht, _chunks=chunks):
                    blk = it * 8 + role * 4 + ht
                    for s, c in reversed(_chunks):
                        if blk >= s:
                            return c[:, (blk - s) * 128 : (blk - s + 1) * 128]

                w1s[j] = w1s_j

            # Consumption-ordered remainder: w2_0 right after expert 0's
            # w1 (mm2_0 follows mm1_0), then x/w1/w2 per expert.
            # (Finer head chunks were tried and regress: every extra
            # dma_start adds ~0.6us of serial DGE config at the stream
            # head, which outweighs the tighter arrival tracking.)
            w2t[0] = load_w2(0)
            for j in range(1, EPC):
                load_rest(j, (16, 16, 16, 16))
                w2t[j] = load_w2(j)

            # --- Compute: grouped SwiGLU MLP per expert. ---
            for j in range(EPC):
                m = int(m_pads[j])
                # mm1: out1^T[o, s] = sum_h W1[o, h] * xd[s, h] per o-tile.
                acts = []
                for it in range(IT):
                    pg = psp.tile([128, m], f32, tag="pg", name="pg", bufs=3)
                    pu = psp.tile([128, m], f32, tag="pu", name="pu", bufs=3)
                    for ht in range(HT):
                        nc.tensor.matmul(pg[:], w1s[j](it, 0, ht), xt[j][ht],
                                         start=(ht == 0), stop=(ht == HT - 1))
                    for ht in range(HT):
                        nc.tensor.matmul(pu[:], w1s[j](it, 1, ht), xt[j][ht],
                                         start=(ht == 0), stop=(ht == HT - 1))
                    sg = actp.tile([128, m], f16, tag="sg", name="sg")
                    nc.scalar.activation(sg[:], pg[:],
                                         mybir.ActivationFunctionType.Silu)
                    a = actp.tile([128, m], f16, tag=f"a{it}", name=f"a{it}")
                    nc.vector.tensor_mul(a[:], sg[:], pu[:])
                    acts.append(a)

                # mm2: y^T[h, s] = sum_i W2[h, i] * act[s, i]; psum copied
                # to the fp16 output tile, alternating Vector/Scalar
                # (Scalar takes the last chunk — it's slightly faster and
                # the final store gates the teardown). Stores ride the
                # same SP ring; their FIFO entries queue up behind the
                # remaining input stream, which is harmless.
                yj = resp.tile([128, HT * m], f16, tag=f"y{j}", name=f"y{j}")
                ybase = HT * offs[j]
                for ht2 in range(HT):
                    ps2 = psp.tile([128, m], f32, tag="ps2", name="ps2")
                    for it in range(IT):
                        b2 = (ht2 * 8 + it) * 128
                        nc.tensor.matmul(ps2[:], w2t[j][:, b2 : b2 + 128],
                                         acts[it][:],
                                         start=(it == 0), stop=(it == IT - 1))
                    dst = yj[:, ht2 * m : (ht2 + 1) * m]
                    if j == EPC - 1 and ht2 == HT - 1:
                        # Final chunk: split the psum copy across both
                        # engines — it gates the last store and teardown.
                        h = m // 2
                        nc.vector.tensor_scalar_mul(dst[:, :h], ps2[:, :h],
                                                    1.0)
                        nc.scalar.copy(dst[:, h:], ps2[:, h:])
                    elif ht2 % 2 == 0:
                        nc.vector.tensor_scalar_mul(dst, ps2[:], 1.0)
                    else:
                        nc.scalar.copy(dst, ps2[:])
                    if j == EPC - 1 and ht2 == HT - 1:
                        # Final chunk in two halves so the very last
                        # store transfer is minimal.
                        h = m // 2
                        nc.sync.dma_start(
                            out=y_d[:, ybase + ht2 * m : ybase + ht2 * m + h],
                            in_=dst[:, :h])
                        nc.sync.dma_start(
                            out=y_d[:, ybase + ht2 * m + h
                                    : ybase + (ht2 + 1) * m],
                            in_=dst[:, h:])
                    elif j == EPC - 1:
                        # Last expert: store each quarter as soon as its
                        # copy lands, so the final store is tiny.
                        nc.sync.dma_start(
                            out=y_d[:, ybase + ht2 * m : ybase + (ht2 + 1) * m],
                            in_=dst)
                if j < EPC - 1:
                    nc.sync.dma_start(
                        out=y_d[:, ybase : ybase + HT * m], in_=yj[:])

    nc.finalize()
    return nc


def kernel(hidden_states, top_k_indices, top_k_weights, gate_up_proj,
           down_proj):
    global LAST_RESULTS
    _ensure_profile_hook()
    from concourse.bass_utils import run_bass_kernel_spmd

    hs = np.ascontiguousarray(np.asarray(hidden_states, dtype=np.float32))
    gup = np.asarray(gate_up_proj, dtype=np.float32)
    dwn = np.asarray(down_proj, dtype=np.float32)

    e_flat, w_flat, tok, slots_per_e = _routing(top_k_indices, top_k_weights)
    counts = np.array([len(s) for s in slots_per_e])
    # Load-balance: sort experts by routed count and deal them out in
    # rounds of NCORES — position j on every core handles one expert from
    # round j, so the per-position compile-time pad (the round max) stays
    # as tight as possible. Descending order also puts the smallest
    # expert last, shortening the post-stream mm2 tail.
    sorted_eids = np.argsort(-counts, kind="stable")
    assign = sorted_eids.reshape(EPC, NCORES)  # [position, core]
    m_pads = tuple(
        int(min(CAP, max(128, int(counts[assign[j]].max()))))
        for j in range(EPC))
    offs = [0]
    for m in m_pads:
        offs.append(offs[-1] + m)
    slots = offs[-1]

    if m_pads not in _prog_cache:
        _prog_cache[m_pads] = _build_program(m_pads)
    nc = _prog_cache[m_pads]

    in_maps = []
    core_exps = []
    for c in range(NCORES):
        exps = [int(assign[j, c]) for j in range(EPC)]
        core_exps.append(exps)
        xd = np.zeros((slots, H), np.float32)
        for j, e in enumerate(exps):
            sl = slots_per_e[e]
            xd[offs[j] : offs[j] + len(sl)] = hs[tok[sl]]
        # xdT[p, HT*off_j + ht*m_j + s] = xd[off_j + s, ht*128 + p]
        parts = []
        for j in range(EPC):
            blk = xd[offs[j] : offs[j + 1]]  # [m_j, H]
            parts.append(
                blk.reshape(m_pads[j], HT, 128).transpose(2, 1, 0)
                .reshape(128, HT * m_pads[j]))
        xdT = np.ascontiguousarray(np.concatenate(parts, axis=1)
                                   .astype(np.float16))
        # w1n[j, p, (it*8 + role*4 + ht)*128 + o]
        #   = gate_up[e_j, role*I + it*128 + o, ht*128 + p]
        w1n = np.ascontiguousarray(
            gup[exps].reshape(EPC, 2, IT, 128, HT, 128)
            .transpose(0, 5, 2, 1, 4, 3)
            .astype(np.float16)).reshape(EPC, 128, 64 * 128)
        # w2n[j, p, (ht2*8 + it)*128 + h] = down[e_j, ht2*128 + h, it*128 + p]
        w2n = np.ascontiguousarray(
            dwn[exps].reshape(EPC, HT, 128, IT, 128)
            .transpose(0, 4, 1, 3, 2)
            .astype(np.float16)).reshape(EPC, 128, 32 * 128)
        in_maps.append({"xdT": xdT, "w1n": w1n, "w2n": w2n})

    res = run_bass_kernel_spmd(nc, in_maps, list(range(NCORES)))
    LAST_RESULTS = res

    # Combine: scatter per-slot outputs back to flat (token, k) slots,
    # apply the routing weights, and reduce over the top-k axis.
    y_tk = np.zeros((T * TOPK, H), np.float32)
    for c in range(NCORES):
        yc = res.results[c]["y"]  # [128, HT*slots] fp16; y^T[h, s] blocks
        for j, e in enumerate(core_exps[c]):
            sl = slots_per_e[e]
            blk = (yc[:, HT * offs[j] : HT * offs[j + 1]]
                   .reshape(128, HT, m_pads[j]))
            # y[s, h] with h = ht*128 + p
            y_full = (blk.transpose(2, 1, 0).reshape(m_pads[j], H)[: len(sl)]
                      .astype(np.float32))
            y_tk[sl] = y_full * w_flat[sl][:, None]
    return y_tk.reshape(T, TOPK, H).sum(axis=1)
